# revision 4
# baseline (speedup 1.0000x reference)
"""Bass/Trainium2 kernel for elementwise Bessel J2 (nn_BesselFunction).

Input:  x float32 [64, 1048576], values in [0, 30)
Output: J2(x) float32 [64, 1048576]

Sharding: trivially data-parallel; row-block shard across 8 NeuronCores.
Each core sees a [128, 65536] view of its 8-row slice.

v2 design — 8 custom-DVE passes + 2 ACT (scalar-engine) passes per tile.
The scalar engine's Sin table (trig_and_small set, valid on [-pi, pi])
replaces the 3-instruction DVE sin polynomial, and its Square activation
materializes t=(x/8)^2 for the small branch; both live in ONE table set so
there is a single ACT_TABLE_LOAD for the whole kernel.

Math (per element, fp32):
  small (x < 8):  J2 ~ P8(t), t = (x/8)^2 in [0,1)   (minimax, err 1.6e-6)
    SPA: w = ((c8 t + c7) t + c6) t + c5      [4 coeffs via C3 spill]
    SPB: w = ((w t + c4) t + c3) t + c2
    SPC: sm = select(t < 1, (w t + c1) t + c0, SENT)
  big (x >= 8):   J2 = A(v) * sin(2 pi z),  v = 1/x
    rf  = reciprocal_approx_fast(x)                      (~51 ULP)
    xp  = x + ((G2 u + G1) u + G0) * rf,  u = rf^2       (phase poly)
    z   = m - round(m),  m = xp/(2pi) - 3/8              (magic-number round;
          sin(2 pi z) == sin(xp - 3pi/4 - 2 pi k) exactly, 3pi/4 = 0.375*2pi)
    sb  = ACT Sin(2 pi * z)
    ot  = ((A2 v + A1) v + A0) * sb                      (quad amplitude,
          4.6e-3 rel; well under the 2e-2 gate)
  COMB: out = select(sm == SENT, ot, sm)
Dead lanes (x<8 in the big path) may hold Inf/NaN; never selected.
"""

import os
import sys

import numpy as np

for _p in ("/opt/trn_rl_repo", os.path.expanduser("~/.axon_site/_ro/trn_rl_repo")):
    if os.path.isdir(_p) and _p not in sys.path:
        sys.path.insert(0, _p)

# ---------------------------------------------------------------- constants
# small branch: deg-8 minimax of J2 on [0,8] in t=(x/8)^2 (fit err 1.58e-6)
C_SMALL = (
    1.5722338526528576e-06, 7.99973995364862, -42.65958460017419,
    85.25891172836403, -90.62865053407347, 59.502466606776295,
    -25.6331180340566, 6.982961491496972, -0.9357214769677028,
)
# big branch phase correction g(u) = G0 + G1 u + G2 u^2, theta = x - 3pi/4 + g*r
G0, G1, G2 = 1.8750017212985988, -0.35273547793587845, -1.269081537467201
# quad amplitude fit of sqrt(v)*K(v^2) on v in [1/30.5, 1/8] (4.6e-3 rel)
A0, A1, A2 = 0.07446199438385306, 2.3294927147248323, -5.164277911743105
INV_2PI = 0.15915494309189535
TWO_PI = 6.283185307179586
MAGIC = 12582912.0  # 1.5 * 2^23
SENT = 1.0e30

P = 128
COLS = 65536          # per-core elements / 128 partitions
FREE = 2048           # tile free dim
N_CORES = 8

_CACHE: dict = {}


def _register_custom_ops():
    from concourse import dve_ops
    from concourse.dve_spec import (
        Spec, Src0, Src1, C0, C1, C2, C3, One, sq, eq, select, lower,
        _has_src1, _spill_c3_to_src1,
    )
    from concourse.dve_uop import DveOpSpec

    def register_op(name, spec):
        for op in dve_ops.OPS:
            if op.name == name:
                return op
        row = max(dve_ops._SUB_OPCODE_FOR_NAME.values()) + 1
        assert row < 0x20, "out of custom-DVE opcode rows"
        dve_ops._SUB_OPCODE_FOR_NAME[name] = row
        shas = {}
        for ver in ("v3", "v4"):
            try:
                s = DveOpSpec(name=name, opcode=row, uops=lower(spec, ver=ver),
                              rd1_en=_has_src1(spec))
                shas[ver] = s.sha(ver)
            except Exception:
                if ver == "v3":
                    raise
        op = dve_ops.DveOp(name, spec, subdim=False, uops_sha=shas)
        dve_ops.OPS.append(op)
        dve_ops.CUSTOM_DVE_SPECS[name] = spec
        return op

    ops = {}
    # xp = x + ((G2 u + G1) u + G0) * rf, u = rf^2   [in0=rf, in1=x]
    ops["PHASE"] = register_op("J2_PHASE", Spec(
        body=Src1 + ((C0 * sq(Src0) + C1) * sq(Src0) + C2) * Src0,
        reference=lambda in0, in1, c0, c1, c2:
            in1 + ((c0 * in0 * in0 + c1) * (in0 * in0) + c2) * in0,
    ))
    # z = m - round(m), m = xp*C0 + C1, round via +/- magic   [in0=xp]
    _m = Src0 * C0 + C1
    ops["ZRED"] = register_op("J2_ZRED", Spec(
        body=_m - ((_m + C2) - C2),
        reference=lambda in0, in1, c0, c1, c2: (
            lambda m: m - (np.float32(np.float32(m + np.float32(c2))
                                      - np.float32(c2)))
        )(np.float32(np.float32(in0 * np.float32(c0)) + np.float32(c1))),
    ))
    # ot = ((A2 v + A1) v + A0) * sb   [in0=rf, in1=sb]
    ops["AMPSIN"] = register_op("J2_AMPSIN", Spec(
        body=((C0 * Src0 + C1) * Src0 + C2) * Src1,
        reference=lambda in0, in1, c0, c1, c2:
            ((c0 * in0 + c1) * in0 + c2) * in1,
    ))
    # w = ((C0 t + C1) t + C2) t + C3   [in0=t, C3 spilled to in1]
    ops["SPA4"] = register_op("J2_SPA4", Spec(
        body=_spill_c3_to_src1(((C0 * Src0 + C1) * Src0 + C2) * Src0 + C3),
        reference=lambda in0, in1, c0, c1, c2:
            ((c0 * in0 + c1) * in0 + c2) * in0 + in1,
    ))
    # w = ((w t + C0) t + C1) t + C2   [in0=w, in1=t]
    ops["SPB"] = register_op("J2_SPB", Spec(
        body=((Src0 * Src1 + C0) * Src1 + C1) * Src1 + C2,
        reference=lambda in0, in1, c0, c1, c2:
            ((in0 * in1 + c0) * in1 + c1) * in1 + c2,
    ))
    # sm = select(t < 1, (w t + C0) t + C1, C2)   [in0=w, in1=t]
    ops["SPC"] = register_op("J2_SPC", Spec(
        body=select(Src1 < One, (Src0 * Src1 + C0) * Src1 + C1, C2),
        reference=lambda in0, in1, c0, c1, c2:
            np.where(in1 < 1.0, (in0 * in1 + c0) * in1 + c1, c2),
    ))
    # out = select(sm == SENT, big, sm)   [in0=big, in1=sm]
    ops["COMB"] = register_op("J2_COMB", Spec(
        body=select(eq(Src1, C0), Src0, Src1),
        reference=lambda in0, in1, c0, c1, c2:
            np.where(in1 == c0, in0, in1),
    ))
    return ops


def _build_program(repeat: int = 1, free: int = FREE):
    key = (repeat, free)
    if key in _CACHE:
        return _CACHE[key]

    from contextlib import ExitStack, nullcontext

    import concourse.bacc as bacc
    import concourse.bass as bass
    import concourse.tile as tile
    from concourse import mybir

    ops = _register_custom_ops()
    f32 = mybir.dt.float32
    AF = mybir.ActivationFunctionType
    nt = COLS // free

    nc = bacc.Bacc("TRN2", target_bir_lowering=False, debug=False)
    x_d = nc.dram_tensor("x", [P, COLS], f32, kind="ExternalInput")
    o_d = nc.dram_tensor("out", [P, COLS], f32, kind="ExternalOutput")
    x_ap = x_d.ap()
    o_ap = o_d.ap()

    # [P,1] SBUF constant for SPA4's spilled C3 coefficient
    c3t = nc.alloc_sbuf_tensor("j2-c3-const", [P, 1], f32)
    nc.gpsimd.memset(c3t.ap(), C_SMALL[5])
    nc.all_engine_barrier()
    c3_ap = c3t.ap()

    cd = nc.vector._custom_dve

    with tile.TileContext(nc) as tc, ExitStack() as ctx:
        pools = {}
        for name in ("xt", "tq", "rf", "xp", "z", "sb", "w1", "w2", "sm",
                     "ot", "res"):
            pools[name] = ctx.enter_context(tc.tile_pool(name=name, bufs=2))

        def pt(pool, tag=None):
            return pools[pool].tile([P, free], f32, name=tag or pool,
                                    tag=tag or pool)

        loop_cm = tc.For_i(0, repeat, 1) if repeat > 1 else nullcontext()
        with loop_cm:
          for i in range(nt):
            sl = bass.ts(i, free)
            xt = pt("xt")
            nc.sync.dma_start(xt[:], x_ap[:, sl])

            # ---- small branch input on ACT: t = (x/8)^2 ----
            tq = pt("tq")
            nc.scalar.activation(tq[:], xt[:], AF.Square, bias=0.0, scale=0.125)

            # ---- big branch phase on DVE ----
            rf = pt("rf")
            nc.vector.reciprocal_approx_fast(out=rf[:], in_=xt[:])
            xp = pt("xp")
            cd(ops["PHASE"], out=xp[:], in0=rf[:], in1=xt[:],
               s0=G2, s1=G1, imm2=G0)
            z = pt("z")
            cd(ops["ZRED"], out=z[:], in0=xp[:],
               s0=INV_2PI, s1=-0.375, imm2=MAGIC)

            # ---- sin on ACT (table valid on [-pi, pi]; 2*pi*z is in range)
            sb = pt("sb")
            nc.scalar.activation(sb[:], z[:], AF.Sin, bias=0.0, scale=TWO_PI)

            # ---- small branch poly on DVE (overlaps ACT sin) ----
            w1 = pt("w1")
            cd(ops["SPA4"], out=w1[:], in0=tq[:], in1=c3_ap,
               s0=C_SMALL[8], s1=C_SMALL[7], imm2=C_SMALL[6])
            w2 = pt("w2")
            cd(ops["SPB"], out=w2[:], in0=w1[:], in1=tq[:],
               s0=C_SMALL[4], s1=C_SMALL[3], imm2=C_SMALL[2])
            sm = pt("sm")
            cd(ops["SPC"], out=sm[:], in0=w2[:], in1=tq[:],
               s0=C_SMALL[1], s1=C_SMALL[0], imm2=SENT)

            # ---- big branch amplitude * sin, then combine ----
            ot = pt("ot")
            cd(ops["AMPSIN"], out=ot[:], in0=rf[:], in1=sb[:],
               s0=A2, s1=A1, imm2=A0)
            res = pt("res")
            cd(ops["COMB"], out=res[:], in0=ot[:], in1=sm[:], s0=SENT)

            nc.sync.dma_start(o_ap[:, sl], res[:])

    nc.compile()
    _CACHE[key] = {"nc": nc}
    return _CACHE[key]


def kernel(x: np.ndarray) -> np.ndarray:
    from concourse import bass_utils

    prog = _build_program()
    x = np.asarray(x, dtype=np.float32)
    rows = x.shape[0] // N_CORES
    in_maps = [
        {"x": np.ascontiguousarray(
            x[rows * k: rows * (k + 1)].reshape(P, COLS))}
        for k in range(N_CORES)
    ]
    res = bass_utils.run_bass_kernel_spmd(
        prog["nc"], in_maps, core_ids=list(range(N_CORES)))
    out = np.concatenate(
        [res.results[k]["out"].reshape(rows, -1) for k in range(N_CORES)], axis=0)
    return out.astype(np.float32)
